# revision 1
# baseline (speedup 1.0000x reference)
"""Pairwise cosine similarity  O = (Z/|Z_rows|) @ (Y/|Y_rows|).T  on 8 TRN2 cores.

Sharding: Z rows split across 8 cores (data parallel), Y replicated.
Each core computes a [512, 4096] block of the [4096, 4096] output.

Per-core pipeline (fp32 data, fp32r matmuls = full PE rate):
  1. Load Zc [512, 4096] naturally (one DMA per 128-row subtile); row sumsq
     on the scalar engine (Square activation with accum_out); scale rows by
     1/|z| in place (DVE); PE-transpose (fp32) into an SBUF-resident kxm
     cache [128, 32k, 512m] - the PSUM->SBUF copyback writes float32r,
     satisfying the fp32r rounding rule.
  2. Stream Y in chunks of 256 rows: row sumsq -> scale rows by 1/|y| in
     place on the (otherwise idle) GPSIMD engine; PE-transpose [128,128]
     blocks (fp32), 8 blocks (4 k-tiles x 2 row-subtiles) per 2-bank PSUM
     tile, one batched DVE copyback into the fp32r moving-operand tile
     [128, 4k, 256rows]; matmul-accumulate over 32 k-tiles into 4 PSUM
     banks.
  3. Evict PSUM with a plain DVE copy (norms already folded into inputs),
     DMA the output block on the gpsimd queue.
"""

import contextlib
import os
import sys
import numpy as np

_TRN_REPO = "/opt/trn_rl_repo"
if _TRN_REPO not in sys.path:
    sys.path.insert(0, _TRN_REPO)

import concourse.bacc as bacc
import concourse.mybir as mybir
import concourse.tile as tile
from concourse.bass_utils import run_bass_kernel_spmd
from concourse.masks import make_identity

P = 128
N_CORES = 8
F32 = mybir.dt.float32
F32R = mybir.dt.float32r


def build(bz_core=512, by=4096, feat=4096, n_chunk=256, bench_iters=None):
    """Build + bacc-compile the SPMD program (same program on every core)."""
    assert bz_core % P == 0 and by % n_chunk == 0 and feat % P == 0
    assert n_chunk % P == 0 and (feat // P) % 4 == 0
    m_sub = bz_core // P          # output row sub-tiles (PSUM banks)
    k_tiles = feat // P           # contraction tiles
    n_chunks = by // n_chunk      # Y row chunks
    j_sub = n_chunk // P          # row sub-tiles per Y chunk
    KB = min(4, k_tiles)          # k-tiles batched per transpose-copyback
    ZB = min(8, k_tiles)          # Z-phase k-tile batch

    nc = bacc.Bacc("TRN2", target_bir_lowering=False, debug=False,
                   num_devices=N_CORES)
    if bench_iters is None:
        z = nc.dram_tensor("z", [bz_core, feat], F32, kind="ExternalInput").ap()
        y = nc.dram_tensor("y", [by, feat], F32, kind="ExternalInput").ap()
        o = nc.dram_tensor("o", [bz_core, by], F32, kind="ExternalOutput").ap()
    else:
        # bench mode: no host I/O, garbage-content internal tensors
        z = nc.dram_tensor("zi", [bz_core, feat], F32).ap()
        y = nc.dram_tensor("yi", [by, feat], F32).ap()
        o = nc.dram_tensor("oi", [bz_core, by], F32).ap()
        dummy_in = nc.dram_tensor("dummy_in", [1, 64], F32,
                                  kind="ExternalInput").ap()
        dummy_out = nc.dram_tensor("dummy_out", [1, 64], F32,
                                   kind="ExternalOutput").ap()

    with tile.TileContext(nc) as tc:
        with tc.tile_pool(name="const", bufs=1) as const_pool, \
             tc.tile_pool(name="kxm", bufs=1) as kxm_pool, \
             tc.tile_pool(name="nat", bufs=3) as nat_pool, \
             tc.tile_pool(name="small", bufs=2) as small_pool, \
             tc.tile_pool(name="sq", bufs=2) as sq_pool, \
             tc.tile_pool(name="yt", bufs=3) as yt_pool, \
             tc.tile_pool(name="outs", bufs=3) as out_pool, \
             tc.tile_pool(name="pacc", bufs=1, space="PSUM") as pacc_pool, \
             tc.tile_pool(name="ptr", bufs=2, space="PSUM") as ptr_pool:

            ident32 = const_pool.tile([P, P], F32)
            make_identity(nc, ident32)

            if bench_iters is None:
                _loop = contextlib.nullcontext()
            else:
                _loop = tc.For_i(0, bench_iters, 1)
            with _loop:
                def row_rnorm(nat_ap, rdst):
                    """rdst[p,0] = 1/|row p| for a [P, feat] natural tile.

                    Squares+partial sums on the scalar engine.
                    """
                    parts = small_pool.tile([P, feat // 512], F32, tag="parts")
                    for s in range(feat // 512):
                        sq = sq_pool.tile([P, 512], F32, tag="sqscratch")
                        nc.scalar.activation(
                            sq[:], nat_ap[:, s * 512:(s + 1) * 512],
                            mybir.ActivationFunctionType.Square,
                            accum_out=parts[:, s:s + 1])
                    ss = small_pool.tile([P, 1], F32, tag="ss")
                    nc.vector.reduce_sum(ss[:], parts[:],
                                         axis=mybir.AxisListType.X)
                    std = small_pool.tile([P, 1], F32, tag="std")
                    nc.scalar.sqrt(std[:], ss[:])
                    nc.vector.reciprocal(rdst, std[:])

                # ---- Z phase: norms + prescale + transpose into kxm ----
                # Z is loaded as ynat-shaped halves so the nat pool slots all
                # have one size and Z buffers recycle into Y chunk buffers.
                assert m_sub % j_sub == 0
                zn_tiles = []
                for h in range(m_sub // j_sub):
                    znh = nat_pool.tile([P, j_sub, feat], F32, tag="nat",
                                        name=f"zn{h}")
                    for jj in range(j_sub):
                        j = h * j_sub + jj
                        nc.sync.dma_start(out=znh[:, jj],
                                          in_=z[j * P:(j + 1) * P, :])
                    zn_tiles.append(znh)
                rz = small_pool.tile([P, m_sub], F32, tag="rz")
                for j in range(m_sub):
                    znj = zn_tiles[j // j_sub][:, j % j_sub]
                    row_rnorm(znj, rz[:, j:j + 1])
                    nc.vector.tensor_scalar_mul(znj, znj, rz[:, j:j + 1])
                kxm = kxm_pool.tile([P, k_tiles, bz_core], F32R)
                for j in range(m_sub):
                    znj = zn_tiles[j // j_sub][:, j % j_sub]
                    for k0 in range(0, k_tiles, ZB):
                        pt = ptr_pool.tile([P, 1024], F32, tag="ptp")
                        for i in range(ZB):
                            nc.tensor.transpose(
                                pt[:, i * P:(i + 1) * P],
                                znj[:, (k0 + i) * P:(k0 + i + 1) * P],
                                ident32[:])
                        nc.vector.tensor_copy(
                            kxm[:, k0:k0 + ZB, j * P:(j + 1) * P],
                            pt[:, :ZB * P].rearrange("p (i q) -> p i q", i=ZB))

                # ---- main loop over Y chunks ----
                for c in range(n_chunks):
                    ynat = nat_pool.tile([P, j_sub, feat], F32, tag="nat")
                    for j in range(j_sub):
                        nc.sync.dma_start(
                            out=ynat[:, j],
                            in_=y[c * n_chunk + j * P:
                                  c * n_chunk + (j + 1) * P, :])
                    ry = small_pool.tile([P, j_sub], F32, tag="ry")
                    for j in range(j_sub):
                        row_rnorm(ynat[:, j], ry[:, j:j + 1])
                    # broadcast row of 1/|y|: [1, n_chunk] -> [128, n_chunk]
                    ryrow = small_pool.tile([P, n_chunk], F32, tag="ryrow")
                    for j in range(j_sub):
                        ptt = ptr_pool.tile([P, 1024], F32, tag="ptp",
                                            name="ptt")
                        nc.tensor.transpose(ptt[:1, :P], ry[:, j:j + 1],
                                            ident32[:])
                        nc.vector.tensor_copy(
                            ryrow[:1, j * P:(j + 1) * P], ptt[:1, :P])
                    ryb = small_pool.tile([P, n_chunk], F32, tag="ryb")
                    nc.gpsimd.partition_broadcast(ryb[:], ryrow[:1, :])

                    accs = [pacc_pool.tile([P, n_chunk], F32, tag=f"acc{m}",
                                           name=f"acc{m}")
                            for m in range(m_sub)]
                    for k0 in range(0, k_tiles, KB):
                        yt = yt_pool.tile([P, KB, n_chunk], F32R, tag="yt")
                        pt = ptr_pool.tile([P, 1024], F32, tag="ptp")
                        for i in range(KB):
                            for j in range(j_sub):
                                nc.tensor.transpose(
                                    pt[:, (i * j_sub + j) * P:
                                       (i * j_sub + j + 1) * P],
                                    ynat[:, j, (k0 + i) * P:(k0 + i + 1) * P],
                                    ident32[:])
                        nc.vector.tensor_copy(
                            yt[:], pt[:].rearrange("p (i n) -> p i n", i=KB))
                        for i in range(KB):
                            for m in range(m_sub):
                                nc.tensor.matmul(
                                    accs[m][:],
                                    kxm[:, k0 + i, m * P:(m + 1) * P],
                                    yt[:, i, :],
                                    start=(k0 + i == 0),
                                    stop=(k0 + i == k_tiles - 1))
                    for m in range(m_sub):
                        ob = out_pool.tile([P, n_chunk], F32, tag="ob")
                        nc.vector.tensor_mul(ob[:], accs[m][:], ryb[:])
                        nc.gpsimd.dma_start(
                            out=o[m * P:(m + 1) * P,
                                  c * n_chunk:(c + 1) * n_chunk],
                            in_=ob[:])

            if bench_iters is not None:
                db = const_pool.tile([1, 64], F32, tag="db", name="db")
                nc.sync.dma_start(out=db[:], in_=dummy_in[:])
                nc.vector.tensor_copy(db[:], db[:])
                nc.sync.dma_start(out=dummy_out[:], in_=db[:])

    nc.compile()
    return nc


_CACHE = {}


def _get_compiled():
    if "nc" not in _CACHE:
        _CACHE["nc"] = build()
    return _CACHE["nc"]


def kernel(Z, Y):
    Z = np.ascontiguousarray(np.asarray(Z, dtype=np.float32))
    Y = np.ascontiguousarray(np.asarray(Y, dtype=np.float32))
    bz = Z.shape[0]
    shard = bz // N_CORES
    nc = _get_compiled()
    in_maps = [{"z": Z[i * shard:(i + 1) * shard], "y": Y}
               for i in range(N_CORES)]
    res = run_bass_kernel_spmd(nc, in_maps, list(range(N_CORES)))
    out = np.concatenate([res.results[i]["o"] for i in range(N_CORES)], axis=0)
    return out



# revision 7
# speedup vs baseline: 1.4032x; 1.4032x over previous
"""Pairwise cosine similarity  O = (Z/|Z_rows|) @ (Y/|Y_rows|).T  on 8 TRN2 cores.

Sharding: Z rows split across 8 cores (data parallel), Y replicated.
Each core computes O^T block [4096, 512] (y-major); host transposes back.

v6 structure:
  - inputs are loaded fp32->bf16 by casting gpsimd (SWDGE) DMAs; the whole
    on-chip pipeline is bf16 (PE transpose = 1 cyc/row, DVE 2x copybacks,
    half the SBUF). fp32 PSUM accumulation keeps the dot products accurate.
  - matmul orientation: transposed-Y tiles are the STATIONARY operand
    [128k, 128y]; the SBUF-resident Z^T cache (kxm, bf16) is the MOVING
    operand [128k, 512z] (1 cyc/row, N=512). Output blocks are
    [y-part, z-free] so 1/|y| is a per-partition activation scale on the
    scalar engine; the output is O^T, un-transposed on the host.
  - all (chunk, k0) transpose/matmul batches are software-pipelined with
    LAG=2 so PE never head-of-line blocks on the PSUM->SBUF copyback.
  - PSUM: 2x2 accumulator banks (pacc bufs=2) + 2x1 transpose banks
    (ptr bufs=2). kxm bufs=2 so the next bench iteration's Z phase
    overlaps this iteration's tail chunks.
  - row sumsq = one Square activation with accum_out per 128-row tile.
"""

import contextlib
import os
import sys
import numpy as np

_TRN_REPO = "/opt/trn_rl_repo"
if _TRN_REPO not in sys.path:
    sys.path.insert(0, _TRN_REPO)

import concourse.bacc as bacc
import concourse.mybir as mybir
import concourse.tile as tile
from concourse.bass_utils import run_bass_kernel_spmd
from concourse.masks import make_identity

P = 128
N_CORES = 8
F32 = mybir.dt.float32
BF16 = mybir.dt.bfloat16


def build(bz_core=512, by=4096, feat=4096, n_chunk=256, bench_iters=None):
    """Build + bacc-compile the SPMD program (same program on every core)."""
    assert bz_core % P == 0 and by % n_chunk == 0 and feat % P == 0
    assert n_chunk % P == 0 and (feat // P) % 4 == 0
    m_sub = bz_core // P          # z sub-tiles in the kxm free dim
    k_tiles = feat // P           # contraction tiles
    n_chunks = by // n_chunk      # Y row chunks
    j_sub = n_chunk // P          # y sub-tiles per chunk (= acc banks)
    KB = min(4, k_tiles)          # k-tiles batched per transpose-copyback
    ZB = min(8, k_tiles)          # Z-phase k-tile batch
    n_kb = k_tiles // KB          # k0 batches per chunk
    LAG = 2                       # transpose->matmul software-pipeline depth

    nc = bacc.Bacc("TRN2", target_bir_lowering=False, debug=False,
                   num_devices=N_CORES)
    if bench_iters is None:
        z = nc.dram_tensor("z", [bz_core, feat], F32, kind="ExternalInput").ap()
        y = nc.dram_tensor("y", [by, feat], F32, kind="ExternalInput").ap()
        # o holds this core's O^T block [by, bz_core]
        o = nc.dram_tensor("o", [by, bz_core], F32, kind="ExternalOutput").ap()
    else:
        # bench mode: no host I/O, garbage-content internal tensors
        z = nc.dram_tensor("zi", [bz_core, feat], F32).ap()
        y = nc.dram_tensor("yi", [by, feat], F32).ap()
        o = nc.dram_tensor("oi", [by, bz_core], F32).ap()
        dummy_in = nc.dram_tensor("dummy_in", [1, 64], F32,
                                  kind="ExternalInput").ap()
        dummy_out = nc.dram_tensor("dummy_out", [1, 64], F32,
                                   kind="ExternalOutput").ap()

    with tile.TileContext(nc) as tc:
        with tc.tile_pool(name="const", bufs=1) as const_pool, \
             tc.tile_pool(name="kxm", bufs=2) as kxm_pool, \
             tc.tile_pool(name="nat", bufs=3) as nat_pool, \
             tc.tile_pool(name="small", bufs=2) as small_pool, \
             tc.tile_pool(name="sq", bufs=2) as sq_pool, \
             tc.tile_pool(name="yt", bufs=LAG + 1) as yt_pool, \
             tc.tile_pool(name="outs", bufs=3) as out_pool, \
             tc.tile_pool(name="pacc", bufs=2, space="PSUM") as pacc_pool, \
             tc.tile_pool(name="ptr", bufs=2, space="PSUM") as ptr_pool:

            identf = const_pool.tile([P, P], F32, name="identf")
            make_identity(nc, identf)
            identb = const_pool.tile([P, P], BF16, name="identb")
            nc.vector.tensor_copy(identb[:], identf[:])

            if bench_iters is None:
                _loop = contextlib.nullcontext()
            else:
                _loop = tc.For_i(0, bench_iters, 1)
            with _loop:
                def row_rnorm(nat_ap, rdst):
                    """rdst[p,0] = 1/|row p| for a [P, feat] natural tile.

                    One Square activation with accum_out = full row sumsq;
                    the bf16 elementwise output is scratch (values unused).
                    """
                    sq = sq_pool.tile([P, feat], BF16, tag="sqscratch")
                    ss = small_pool.tile([P, 1], F32, tag="ss")
                    nc.scalar.activation(
                        sq[:], nat_ap,
                        mybir.ActivationFunctionType.Square,
                        accum_out=ss[:])
                    std = small_pool.tile([P, 1], F32, tag="std")
                    nc.scalar.sqrt(std[:], ss[:])
                    nc.vector.reciprocal(rdst, std[:])

                # ---- Z phase: norms + prescale + transpose into kxm ----
                # Z is loaded as ynat-shaped halves so the nat pool slots all
                # have one size and Z buffers recycle into Y chunk buffers.
                assert m_sub % j_sub == 0
                zn_tiles = []
                for h in range(m_sub // j_sub):
                    znh = nat_pool.tile([P, j_sub, feat], BF16, tag="nat",
                                        name=f"zn{h}")
                    for jj in range(j_sub):
                        j = h * j_sub + jj
                        nc.gpsimd.dma_start(out=znh[:, jj],
                                            in_=z[j * P:(j + 1) * P, :])
                    zn_tiles.append(znh)
                rz = small_pool.tile([P, m_sub], F32, tag="rz")
                for j in range(m_sub):
                    znj = zn_tiles[j // j_sub][:, j % j_sub]
                    row_rnorm(znj, rz[:, j:j + 1])
                    nc.vector.tensor_scalar_mul(znj, znj, rz[:, j:j + 1])
                kxm = kxm_pool.tile([P, k_tiles, bz_core], BF16)
                for k0 in range(0, k_tiles, ZB):
                    for j in range(m_sub):
                        znj = zn_tiles[j // j_sub][:, j % j_sub]
                        pt = ptr_pool.tile([P, 1024], BF16, tag="ptp")
                        for i in range(ZB):
                            nc.tensor.transpose(
                                pt[:, i * P:(i + 1) * P],
                                znj[:, (k0 + i) * P:(k0 + i + 1) * P],
                                identb[:])
                        nc.vector.tensor_copy(
                            kxm[:, k0:k0 + ZB, j * P:(j + 1) * P],
                            pt[:, :ZB * P].rearrange("p (i q) -> p i q", i=ZB))

                # ---- main loop: software-pipelined (chunk, k0) batches ----
                n_b = n_chunks * n_kb
                ynats = {}   # c -> ynat tile
                rys = {}     # c -> ry tile
                accs = {}    # c -> [j_sub acc tiles]
                yts = {}     # b -> yt tile

                def start_chunk(c):
                    ynat = nat_pool.tile([P, j_sub, feat], BF16, tag="nat")
                    for j in range(j_sub):
                        nc.gpsimd.dma_start(
                            out=ynat[:, j],
                            in_=y[c * n_chunk + j * P:
                                  c * n_chunk + (j + 1) * P, :])
                    ry = small_pool.tile([P, j_sub], F32, tag="ry")
                    for j in range(j_sub):
                        row_rnorm(ynat[:, j], ry[:, j:j + 1])
                    ynats[c] = ynat
                    rys[c] = ry
                    accs[c] = [pacc_pool.tile([P, bz_core], F32,
                                              tag=f"acc{j}", name=f"acc{j}")
                               for j in range(j_sub)]

                def emit_transposes(b):
                    c, kb = divmod(b, n_kb)
                    k0 = kb * KB
                    ynat = ynats[c]
                    pt = ptr_pool.tile([P, KB * j_sub * P], BF16, tag="ptp")
                    for i in range(KB):
                        for j in range(j_sub):
                            nc.tensor.transpose(
                                pt[:, (i * j_sub + j) * P:
                                   (i * j_sub + j + 1) * P],
                                ynat[:, j, (k0 + i) * P:(k0 + i + 1) * P],
                                identb[:])
                    yt = yt_pool.tile([P, KB * j_sub * P], BF16, tag="yt")
                    nc.vector.tensor_copy(yt[:], pt[:])
                    yts[b] = yt
                    if kb == n_kb - 1:
                        del ynats[c]

                def emit_matmuls(b):
                    c, kb = divmod(b, n_kb)
                    k0 = kb * KB
                    yt = yts.pop(b)
                    for i in range(KB):
                        for j in range(j_sub):
                            nc.tensor.matmul(
                                accs[c][j][:],
                                yt[:, (i * j_sub + j) * P:
                                   (i * j_sub + j + 1) * P],
                                kxm[:, k0 + i, :],
                                start=(k0 + i == 0),
                                stop=(k0 + i == k_tiles - 1))
                    if kb == n_kb - 1:
                        evict_chunk(c)

                def evict_chunk(c):
                    ry = rys.pop(c)
                    for j in range(j_sub):
                        ob = out_pool.tile([P, bz_core], F32, tag="ob")
                        nc.scalar.activation(
                            ob[:], accs[c][j][:],
                            mybir.ActivationFunctionType.Copy,
                            scale=ry[:, j:j + 1])
                        nc.sync.dma_start(
                            out=o[c * n_chunk + j * P:
                                  c * n_chunk + (j + 1) * P, :],
                            in_=ob[:])
                    del accs[c]

                for b in range(n_b + LAG):
                    if b < n_b:
                        c, kb = divmod(b, n_kb)
                        if kb == 0:
                            start_chunk(c)
                        emit_transposes(b)
                    if b >= LAG:
                        emit_matmuls(b - LAG)

            if bench_iters is not None:
                db = const_pool.tile([1, 64], F32, tag="db", name="db")
                nc.sync.dma_start(out=db[:], in_=dummy_in[:])
                nc.vector.tensor_copy(db[:], db[:])
                nc.sync.dma_start(out=dummy_out[:], in_=db[:])

    nc.compile()
    return nc


_CACHE = {}


def _get_compiled():
    if "nc" not in _CACHE:
        _CACHE["nc"] = build()
    return _CACHE["nc"]


def kernel(Z, Y):
    Z = np.ascontiguousarray(np.asarray(Z, dtype=np.float32))
    Y = np.ascontiguousarray(np.asarray(Y, dtype=np.float32))
    bz = Z.shape[0]
    shard = bz // N_CORES
    nc = _get_compiled()
    in_maps = [{"z": Z[i * shard:(i + 1) * shard], "y": Y}
               for i in range(N_CORES)]
    res = run_bass_kernel_spmd(nc, in_maps, list(range(N_CORES)))
    # each core returns O^T block [by, shard]; stitch + transpose back
    out_t = np.concatenate([res.results[i]["o"] for i in range(N_CORES)],
                           axis=1)
    return np.ascontiguousarray(out_t.T)
